# revision 1
# baseline (speedup 1.0000x reference)
# GNN mean-aggregation kernel for Trainium2 (8 NeuronCores, SPMD).
#
# Computes: out[i] = (1/deg_i) * sum_{(i,j) in E} (x[j] @ W + b)
# using the algebraic identity  out = inv_deg * (A @ x) @ W + b*mask,
# so the dense linear layer runs on the 100k aggregated rows instead of
# per-edge features.
#
# Sharding: destination nodes (and their incoming edge rows -- `row` is
# sorted) are split contiguously across 8 cores; x and W are replicated,
# so no collectives are needed.
#
# Per-core pipeline:
#   1. dma_gather (GPSIMD SWDGE) fetches x[col] rows (512B each) from HBM
#      in 1024-index calls.  int16 gather indices only span 32k rows, so x
#      is addressed in 4 chunks of 25k rows and edges are host-binned by
#      (dest-tile, chunk), padded to a fixed per-bin column count so the
#      single SPMD instruction stream fits every core.
#   2. DVE builds one-hot segment matrices S^T[e,d] = (rel[e]==d) from
#      host-provided relative-dest values via tensor_tensor(is_equal).
#   3. PE accumulates AGG^T = sum_j M_j^T @ S^T_j in PSUM per 128-dest
#      tile, then OUT^T = W^T @ AGG^T + b (x) deg  (rank-1 bias matmul).
#   4. DVE scales by inv_deg along the dest axis; DMA writes OUT^T.
# Host post-processing transposes and concatenates the per-core outputs.

import math

import numpy as np

P = 128
F = 128


class _Cfg:
    def __init__(self, n_nodes, n_cores, n_chunks, group_tiles=8):
        self.NN = n_nodes
        self.NCORES = n_cores
        self.NDEST = n_nodes // n_cores
        self.NT = math.ceil(self.NDEST / P)
        self.NCH = n_chunks
        self.CH = math.ceil(n_nodes / n_chunks)
        assert self.CH <= 32768
        self.G = group_tiles


CFG = _Cfg(100000, 8, 4)

_BUILD_CACHE = {}


def _host_prep(cfg, x, row, col, W, b):
    NN, NCORES, NDEST, NT, NCH, CH = (
        cfg.NN, cfg.NCORES, cfg.NDEST, cfg.NT, cfg.NCH, cfg.CH)
    NE = row.shape[0]
    row = np.asarray(row).astype(np.int64)
    col = np.asarray(col).astype(np.int64)
    x = np.ascontiguousarray(np.asarray(x, dtype=np.float32))
    W = np.ascontiguousarray(np.asarray(W, dtype=np.float32))
    b = np.asarray(b, dtype=np.float32)

    deg = np.bincount(row, minlength=NN).astype(np.float32)
    invdeg = np.where(deg > 0, 1.0 / np.maximum(deg, 1.0), 0.0).astype(np.float32)

    core = row // NDEST
    r_in_core = row % NDEST
    chunk = col // CH
    idx16 = (col % CH).astype(np.int16)

    # Natural (contiguous) dest->tile assignment unless some (tile, chunk)
    # bin would push C_sub above 9 columns; then greedily rebalance.
    nat_tile = r_in_core // P
    nat_key = (core * NT + nat_tile) * NCH + chunk
    nat_max = np.bincount(nat_key, minlength=NCORES * NT * NCH).max()
    if nat_max <= 9 * P:
        perm = np.tile(np.arange(NDEST, dtype=np.int64)[None, :], (NCORES, 1))
        tilei = nat_tile
        rel = (r_in_core % P).astype(np.float32)
        return _host_prep_finish(
            cfg, x, W, b, deg, invdeg, core, chunk, idx16, tilei, rel, perm)
    # perm[core, d_local] = permuted position (tile*128 + slot).
    perm = np.zeros((NCORES, NDEST), np.int64)
    for c in range(NCORES):
        cnt = np.zeros((NDEST, NCH), np.int32)
        np.add.at(cnt, (r_in_core[core == c], chunk[core == c]), 1)
        order_d = np.argsort(-cnt.max(axis=1), kind="stable")
        sums = np.zeros((NT, NCH), np.int32)
        counts = np.zeros(NT, np.int32)
        pos = np.empty(NDEST, np.int64)
        big = np.int32(1 << 30)
        for d in order_d:
            newmax = np.maximum(sums, cnt[d]).max(axis=1)
            t = int(np.argmin(np.where(counts < P, newmax, big)))
            pos[d] = t * P + counts[t]
            counts[t] += 1
            sums[t] += cnt[d]
        perm[c] = pos
    tilei = perm[core, r_in_core] // P
    rel = (perm[core, r_in_core] % P).astype(np.float32)
    return _host_prep_finish(
        cfg, x, W, b, deg, invdeg, core, chunk, idx16, tilei, rel, perm)


def _host_prep_finish(cfg, x, W, b, deg, invdeg, core, chunk, idx16,
                      tilei, rel, perm):
    NN, NCORES, NDEST, NT, NCH, CH = (
        cfg.NN, cfg.NCORES, cfg.NDEST, cfg.NT, cfg.NCH, cfg.CH)
    NE = core.shape[0]
    bin_key = (core * NT + tilei) * NCH + chunk
    nbins = NCORES * NT * NCH
    counts = np.bincount(bin_key, minlength=nbins)
    C_sub = max(1, int(math.ceil(counts.max() / P)))
    SLOT = C_sub * P

    order = np.argsort(bin_key, kind="stable")
    sk = bin_key[order]
    starts = np.concatenate([[0], np.cumsum(counts)[:-1]])
    rank = np.arange(NE, dtype=np.int64) - starts[sk]
    pos = sk * SLOT + rank

    TOT = nbins * SLOT
    idx_pad = np.zeros(TOT, np.int16)
    rel_pad = np.full(TOT, -1.0, np.float32)
    idx_pad[pos] = idx16[order]
    rel_pad[pos] = rel[order]
    idx_pad = idx_pad.reshape(NCORES, NT, NCH, SLOT)
    rel_pad = rel_pad.reshape(NCORES, NT, NCH, C_sub, P)

    groups = [(t0, min(t0 + cfg.G, NT)) for t0 in range(0, NT, cfg.G)]

    iota2 = np.tile(np.arange(P, dtype=np.float32)[None, :], (P, 1))
    brow = b[None, :]

    in_maps = []
    for c in range(NCORES):
        # gather-call index stream: per (group, chunk), wrapped per <=1024-idx call
        wrapped_parts = []
        for (t0, t1) in groups:
            for ch in range(NCH):
                seq = idx_pad[c, t0:t1, ch].reshape(-1)
                for k0 in range(0, len(seq), 1024):
                    seg = seq[k0:k0 + 1024]
                    wrapped_parts.append(
                        np.tile(seg.reshape(-1, 16).T, (8, 1)))
        idx_t = np.concatenate(wrapped_parts, axis=1)

        rel_t = np.ascontiguousarray(
            rel_pad[c].transpose(3, 0, 1, 2).reshape(P, NT * NCH * C_sub))

        dsl = slice(c * NDEST, (c + 1) * NDEST)
        ivc = np.zeros(NT * P, np.float32)
        ivc[perm[c]] = invdeg[dsl]
        dgc = np.zeros(NT * P, np.float32)
        dgc[perm[c]] = deg[dsl]

        in_maps.append({
            "x": x,
            "idxs": np.ascontiguousarray(idx_t),
            "rel": rel_t,
            "invdeg": np.ascontiguousarray(np.tile(ivc[None, :], (P, 1))),
            "degr": dgc[None, :],
            "w": W,
            "brow": brow,
            "iota2": iota2,
        })
    return C_sub, in_maps, perm


def _build(cfg, C_sub, repeat, parts=("gather", "onehot", "mm")):
    import concourse.mybir as mybir
    import concourse.tile as tile
    from concourse import bacc

    f32 = mybir.dt.float32
    i16 = mybir.dt.int16
    eq = mybir.AluOpType.is_equal
    mult = mybir.AluOpType.mult

    NT, NCH, CH, G = cfg.NT, cfg.NCH, cfg.CH, cfg.G
    C_tot = NCH * C_sub
    IDXW = NT * C_tot * P // 16

    nc = bacc.Bacc("TRN2", debug=False, num_swdge_queues=4)
    x_d = nc.dram_tensor("x", [cfg.NN, F], f32, kind="ExternalInput")
    idx_d = nc.dram_tensor("idxs", [P, IDXW], i16, kind="ExternalInput")
    rel_d = nc.dram_tensor("rel", [P, NT * C_tot], f32, kind="ExternalInput")
    invdeg_d = nc.dram_tensor("invdeg", [P, NT * P], f32, kind="ExternalInput")
    deg_d = nc.dram_tensor("degr", [1, NT * P], f32, kind="ExternalInput")
    w_d = nc.dram_tensor("w", [F, F], f32, kind="ExternalInput")
    b_d = nc.dram_tensor("brow", [1, F], f32, kind="ExternalInput")
    iota_d = nc.dram_tensor("iota2", [P, P], f32, kind="ExternalInput")
    out_d = nc.dram_tensor("outT", [P, NT * P], f32, kind="ExternalOutput")

    groups = [(t0, min(t0 + G, NT)) for t0 in range(0, NT, G)]
    x_ap = x_d.ap()

    with tile.TileContext(nc) as tc:
        with (
            tc.tile_pool(name="const", bufs=1) as constp,
            tc.tile_pool(name="reg", bufs=2) as regionp,
            tc.tile_pool(name="st", bufs=4) as stp,
            tc.tile_pool(name="idx", bufs=2) as idxp,
            tc.tile_pool(name="small", bufs=4) as smallp,
            tc.tile_pool(name="grp", bufs=2) as grpp,
            tc.tile_pool(name="acc", bufs=8, space="PSUM") as accp,
        ):
            w_sb = constp.tile([F, F], f32)
            nc.sync.dma_start(w_sb[:], w_d.ap())
            b_sb = constp.tile([1, F], f32)
            nc.sync.dma_start(b_sb[:], b_d.ap())
            iota_sb = constp.tile([P, P], f32)
            nc.sync.dma_start(iota_sb[:], iota_d.ap())
            rel_sb = constp.tile([P, NT * C_tot], f32)
            nc.sync.dma_start(rel_sb[:], rel_d.ap())

            def body(_iv=None):
                idx_off = 0
                qn = 0
                for (t0, t1) in groups:
                    gt = t1 - t0
                    invdeg_g = grpp.tile([P, gt * P], f32, tag="invdeg")
                    nc.sync.dma_start(
                        invdeg_g[:], invdeg_d.ap()[:, t0 * P:t1 * P])
                    deg_g = grpp.tile([1, gt * P], f32, tag="deg")
                    nc.sync.dma_start(deg_g[:], deg_d.ap()[:, t0 * P:t1 * P])
                    accs = [
                        accp.tile([P, P], f32, tag="acc", name=f"acc{t0}_{k}")
                        for k in range(gt)
                    ]
                    for c in range(NCH):
                        ncols = gt * C_sub
                        reg = regionp.tile([P, ncols, P], f32, tag="reg")
                        idxt = idxp.tile([P, ncols * 8], i16, tag="idx")
                        nc.sync.dma_start(
                            idxt[:], idx_d.ap()[:, idx_off:idx_off + ncols * 8])
                        idx_off += ncols * 8
                        for k0 in range(0, ncols, 8) if "gather" in parts else []:
                            kc = min(8, ncols - k0)
                            L = kc * P
                            nc.gpsimd.dma_gather(
                                out_ap=reg[:, k0:k0 + kc, :],
                                in_ap=x_ap[c * CH:min((c + 1) * CH, cfg.NN), :],
                                idxs_ap=idxt[:, k0 * 8:k0 * 8 + kc * 8],
                                num_idxs=L,
                                num_idxs_reg=L,
                                elem_size=F,
                                queue_num=qn % 4,
                            )
                            qn += 1
                        for ti in range(gt) if ("onehot" in parts or "mm" in parts) else []:
                            t = t0 + ti
                            st = stp.tile([P, C_sub, P], f32, tag="st")
                            rel_sl = rel_sb[:, (t * NCH + c) * C_sub:
                                            (t * NCH + c + 1) * C_sub]
                            if "onehot" in parts:
                                nc.vector.tensor_tensor(
                                    out=st[:],
                                    in0=iota_sb[:].unsqueeze(1).to_broadcast(
                                        [P, C_sub, P]),
                                    in1=rel_sl.to_broadcast([P, C_sub, P]),
                                    op=eq,
                                )
                            accap = accs[ti][:]
                            for j in range(C_sub) if "mm" in parts else []:
                                nc.tensor.matmul(
                                    out=accap,
                                    lhsT=reg[:, ti * C_sub + j, :],
                                    rhs=st[:, j, :],
                                    start=(c == 0 and j == 0),
                                    stop=(c == NCH - 1 and j == C_sub - 1),
                                )
                    for ti in range(gt) if "mm" in parts else []:
                        t = t0 + ti
                        accap = accs[ti][:]
                        aggT = smallp.tile([P, P], f32, tag="agg")
                        nc.scalar.copy(aggT[:], accap)
                        # reuse the same PSUM bank for the output matmul
                        nc.tensor.matmul(out=accap, lhsT=w_sb[:], rhs=aggT[:],
                                         start=True, stop=False)
                        nc.tensor.matmul(out=accap, lhsT=b_sb[:1, :],
                                         rhs=deg_g[:1, ti * P:(ti + 1) * P],
                                         start=False, stop=True)
                        osb = smallp.tile([P, P], f32, tag="osb")
                        nc.vector.tensor_tensor(
                            out=osb[:], in0=accap,
                            in1=invdeg_g[:, ti * P:(ti + 1) * P], op=mult)
                        nc.sync.dma_start(
                            out_d.ap()[:, t * P:(t + 1) * P], osb[:])

            if repeat == 1:
                body()
            else:
                with tc.For_i(0, repeat, 1) as iv:
                    body(iv)

    nc.compile()
    return nc


def _run(cfg, x, row, col, W, b, repeat=1, core_ids=None):
    from concourse import bass_utils

    C_sub, in_maps, perm = _host_prep(cfg, x, row, col, W, b)
    key = (cfg.NN, cfg.NCORES, C_sub, repeat)
    if key not in _BUILD_CACHE:
        _BUILD_CACHE[key] = _build(cfg, C_sub, repeat)
    nc = _BUILD_CACHE[key]
    if core_ids is None:
        core_ids = list(range(cfg.NCORES))
    res = bass_utils.run_bass_kernel_spmd(nc, in_maps, core_ids=core_ids)
    outs = []
    for c in range(len(core_ids)):
        outT = res.results[c]["outT"]
        outs.append(outT.T[perm[c]])
    return np.concatenate(outs, axis=0)


def kernel(x, row, col, W, b):
    return _run(CFG, x, row, col, W, b, repeat=1)



# revision 8
# speedup vs baseline: 1.3872x; 1.3872x over previous
# GNN mean-aggregation kernel for Trainium2 (8 NeuronCores, SPMD).
#
# Computes: out[i] = (1/deg_i) * sum_{(i,j) in E} (x[j] @ W + b)
# using the identity  out = inv_deg * ((A @ x) @ W + b (x) deg),
# so the dense linear layer runs on the 100k aggregated rows instead of
# per-edge features.
#
# Sharding: destination nodes (and their incoming edge rows -- `row` is
# sorted) are split contiguously across 8 cores; x and W are replicated,
# so no collectives are needed.
#
# v2 layout (all fp16 on the wide paths; PSUM accumulates fp32):
#   1. dma_gather (GPSIMD SWDGE) fetches x_fp16[col] rows (256B each) from
#      HBM, one call per (8-tile dest group, 25k-src chunk) to amortize the
#      ~1us fixed SWDGE cost.  int16 gather indices only span 32k rows, so
#      x is addressed in 4 chunks of 25k rows and edges are host-binned by
#      (dest-tile, chunk), padded to C_sub*128 per bin.
#   2. DVE builds one-hot segment matrices S[e, d, j] = (d == rel[e, j]) in
#      fp16 with the block index j packed LAST so the 16-bit 2x DVE mode
#      applies (one instruction per group-chunk).
#   3. PE accumulates AGG^T = sum_j M_j^T @ S_j in PSUM per 128-dest tile,
#      then OUT^T = W^T @ AGG^T + b (x) deg  (rank-1 bias matmul).
#   4. OUT^T tiles are DMA'd straight from PSUM to HBM in fp32.
# Host post-processing transposes, un-permutes, and scales by inv_deg.

import math

import numpy as np

P = 128
F = 128


class _Cfg:
    def __init__(self, n_nodes, n_cores, n_chunks, group_tiles=8):
        self.NN = n_nodes
        self.NCORES = n_cores
        self.NDEST = n_nodes // n_cores
        self.NT = math.ceil(self.NDEST / P)
        self.NCH = n_chunks
        self.CH = math.ceil(n_nodes / n_chunks)
        assert self.CH <= 32768
        self.G = group_tiles


CFG = _Cfg(100000, 8, 4)

_BUILD_CACHE = {}


def _host_prep(cfg, x, row, col, W, b):
    NN, NCORES, NDEST, NT, NCH, CH = (
        cfg.NN, cfg.NCORES, cfg.NDEST, cfg.NT, cfg.NCH, cfg.CH)
    NE = row.shape[0]
    row = np.asarray(row).astype(np.int64)
    col = np.asarray(col).astype(np.int64)
    x = np.ascontiguousarray(np.asarray(x, dtype=np.float32))
    W = np.ascontiguousarray(np.asarray(W, dtype=np.float32))
    b = np.asarray(b, dtype=np.float32)

    deg = np.bincount(row, minlength=NN).astype(np.float32)
    invdeg = np.where(deg > 0, 1.0 / np.maximum(deg, 1.0), 0.0).astype(np.float32)

    core = row // NDEST
    r_in_core = row % NDEST
    chunk = col // CH
    idx16 = (col % CH).astype(np.int16)

    # Natural (contiguous) dest->tile assignment unless some (tile, chunk)
    # bin would push C_sub above 9 columns; then greedily rebalance.
    nat_tile = r_in_core // P
    nat_key = (core * NT + nat_tile) * NCH + chunk
    nat_max = np.bincount(nat_key, minlength=NCORES * NT * NCH).max()
    if nat_max <= 9 * P:
        perm = np.tile(np.arange(NDEST, dtype=np.int64)[None, :], (NCORES, 1))
        tilei = nat_tile
        rel = (r_in_core % P).astype(np.float32)
        return _host_prep_finish(
            cfg, x, W, b, deg, invdeg, core, chunk, idx16, tilei, rel, perm)
    # perm[core, d_local] = permuted position (tile*128 + slot).
    perm = np.zeros((NCORES, NDEST), np.int64)
    for c in range(NCORES):
        cnt = np.zeros((NDEST, NCH), np.int32)
        np.add.at(cnt, (r_in_core[core == c], chunk[core == c]), 1)
        order_d = np.argsort(-cnt.max(axis=1), kind="stable")
        sums = np.zeros((NT, NCH), np.int32)
        counts = np.zeros(NT, np.int32)
        pos = np.empty(NDEST, np.int64)
        big = np.int32(1 << 30)
        for d in order_d:
            newmax = np.maximum(sums, cnt[d]).max(axis=1)
            t = int(np.argmin(np.where(counts < P, newmax, big)))
            pos[d] = t * P + counts[t]
            counts[t] += 1
            sums[t] += cnt[d]
        perm[c] = pos
    tilei = perm[core, r_in_core] // P
    rel = (perm[core, r_in_core] % P).astype(np.float32)
    return _host_prep_finish(
        cfg, x, W, b, deg, invdeg, core, chunk, idx16, tilei, rel, perm)


def _host_prep_finish(cfg, x, W, b, deg, invdeg, core, chunk, idx16,
                      tilei, rel, perm):
    NN, NCORES, NDEST, NT, NCH, CH, G = (
        cfg.NN, cfg.NCORES, cfg.NDEST, cfg.NT, cfg.NCH, cfg.CH, cfg.G)
    NE = core.shape[0]
    bin_key = (core * NT + tilei) * NCH + chunk
    nbins = NCORES * NT * NCH
    counts = np.bincount(bin_key, minlength=nbins)
    C_sub = max(1, int(math.ceil(counts.max() / P)))
    SLOT = C_sub * P

    order = np.argsort(bin_key, kind="stable")
    sk = bin_key[order]
    starts = np.concatenate([[0], np.cumsum(counts)[:-1]])
    rank = np.arange(NE, dtype=np.int64) - starts[sk]
    pos = sk * SLOT + rank

    TOT = nbins * SLOT
    idx_pad = np.zeros(TOT, np.int16)
    rel_pad = np.full(TOT, -1.0, np.float32)
    idx_pad[pos] = idx16[order]
    rel_pad[pos] = rel[order]
    # [core, tile, chunk, C_sub, P] (slot s = (j, p) = (s // P, s % P))
    idx_pad = idx_pad.reshape(NCORES, NT, NCH, SLOT)
    rel_pad = rel_pad.reshape(NCORES, NT, NCH, C_sub, P)

    groups = [(t0, min(t0 + G, NT)) for t0 in range(0, NT, G)]

    x16 = x.astype(np.float16)
    W16 = W.astype(np.float16)
    b16 = b.astype(np.float16)[None, :]
    iota3 = np.tile(
        np.arange(P, dtype=np.float16)[None, :, None], (P, 1, G * C_sub)
    ).reshape(P, P * G * C_sub)

    in_maps = []
    for c in range(NCORES):
        # gather-call index stream: one call per (group, chunk), wrapped in
        # 16 partitions and replicated x8 down the partition dim.
        wrapped_parts = []
        for (t0, t1) in groups:
            for ch in range(NCH):
                seq = idx_pad[c, t0:t1, ch].reshape(-1)
                wrapped_parts.append(np.tile(seq.reshape(-1, 16).T, (8, 1)))
        idx_t = np.concatenate(wrapped_parts, axis=1)

        # rel2[p, ch, t*C_sub + j] = rel_pad[c, t, ch, j, p]
        rel2 = np.ascontiguousarray(
            rel_pad[c].transpose(3, 1, 0, 2).reshape(P, NCH, NT * C_sub)
        ).astype(np.float16).reshape(P, NCH * NT * C_sub)

        dsl = slice(c * NDEST, (c + 1) * NDEST)
        dgc = np.zeros(NT * P, np.float32)
        dgc[perm[c]] = deg[dsl]

        in_maps.append({
            "x": x16,
            "idxs": np.ascontiguousarray(idx_t),
            "rel": rel2,
            "degr": dgc.astype(np.float16)[None, :],
            "w": W16,
            "brow": b16,
            "iota3": iota3,
        })
    return C_sub, in_maps, perm, invdeg


def _build(cfg, C_sub, repeat, onehot="edj", call="wrap"):
    import concourse.mybir as mybir
    import concourse.tile as tile
    from concourse import bacc

    f32 = mybir.dt.float32
    f16 = mybir.dt.float16
    i16 = mybir.dt.int16
    eq = mybir.AluOpType.is_equal

    NT, NCH, CH, G = cfg.NT, cfg.NCH, cfg.CH, cfg.G
    GC = G * C_sub
    IDXW = NT * NCH * C_sub * P // 16

    # sub = max indices per dma_gather call; the SWDGE descriptor ring
    # (dynamic_dma_scratch_size // 16 entries) must hold a full call.
    if call == "wrap":
        sub = 1024
    elif call == "big":
        sub = GC * P
    else:
        sub = int(call[1:])
    scratch = max(16384, 16 * sub)

    nc = bacc.Bacc("TRN2", debug=False, num_swdge_queues=4,
                   dynamic_dma_scratch_size=scratch)
    x_d = nc.dram_tensor("x", [cfg.NN, F], f16, kind="ExternalInput")
    idx_d = nc.dram_tensor("idxs", [P, IDXW], i16, kind="ExternalInput")
    rel_d = nc.dram_tensor("rel", [P, NCH * NT * C_sub], f16,
                           kind="ExternalInput")
    deg_d = nc.dram_tensor("degr", [1, NT * P], f16, kind="ExternalInput")
    w_d = nc.dram_tensor("w", [F, F], f16, kind="ExternalInput")
    b_d = nc.dram_tensor("brow", [1, F], f16, kind="ExternalInput")
    iota_d = nc.dram_tensor("iota3", [P, P * GC], f16, kind="ExternalInput")
    out_d = nc.dram_tensor("outT", [P, NT * P], f32, kind="ExternalOutput")

    groups = [(t0, min(t0 + G, NT)) for t0 in range(0, NT, G)]
    x_ap = x_d.ap()

    with tile.TileContext(nc) as tc:
        with (
            tc.tile_pool(name="const", bufs=1) as constp,
            tc.tile_pool(name="reg", bufs=3) as regionp,
            tc.tile_pool(name="st", bufs=2) as stp,
            tc.tile_pool(name="idx", bufs=3) as idxp,
            tc.tile_pool(name="small", bufs=4) as smallp,
            tc.tile_pool(name="acc", bufs=8, space="PSUM") as accp,
        ):
            w_sb = constp.tile([F, F], f16)
            nc.sync.dma_start(w_sb[:], w_d.ap())
            b_sb = constp.tile([1, F], f16)
            nc.sync.dma_start(b_sb[:], b_d.ap())
            iota_sb = constp.tile([P, P, GC], f16)
            nc.sync.dma_start(iota_sb[:], iota_d.ap())
            rel_sb = constp.tile([P, NCH, NT * C_sub], f16)
            nc.sync.dma_start(rel_sb[:], rel_d.ap())
            deg_sb = constp.tile([1, NT * P], f16)
            nc.sync.dma_start(deg_sb[:], deg_d.ap())

            def body(_iv=None):
                idx_off = 0
                qn = 0
                for (t0, t1) in groups:
                    gt = t1 - t0
                    gc = gt * C_sub
                    accs = [
                        accp.tile([P, P], f32, tag="acc", name=f"acc{t0}_{k}")
                        for k in range(gt)
                    ]
                    for c in range(NCH):
                        L = gc * P
                        reg = regionp.tile([P, gc, P], f16, tag="reg")
                        idxt = idxp.tile([P, gc * 8], i16, tag="idx")
                        nc.sync.dma_start(
                            idxt[:], idx_d.ap()[:, idx_off:idx_off + gc * 8])
                        idx_off += gc * 8
                        in_ap = x_ap[c * CH:min((c + 1) * CH, cfg.NN), :]
                        kstep = sub // P
                        for k0 in range(0, gc, kstep):
                            kc = min(kstep, gc - k0)
                            Ls = kc * P
                            nc.gpsimd.dma_gather(
                                out_ap=reg[:, k0:k0 + kc, :],
                                in_ap=in_ap,
                                idxs_ap=idxt[:, k0 * 8:k0 * 8 + kc * 8],
                                num_idxs=Ls,
                                num_idxs_reg=Ls,
                                elem_size=F,
                                queue_num=qn % 4,
                            )
                            qn += 1
                        if onehot == "edj":
                            st = stp.tile([P, P, gc], f16, tag="st")
                            nc.vector.tensor_tensor(
                                out=st[:],
                                in0=iota_sb[:, :, :gc],
                                in1=rel_sb[:, c, t0 * C_sub:t0 * C_sub + gc]
                                    .unsqueeze(1).to_broadcast([P, P, gc]),
                                op=eq,
                            )
                            for ti in range(gt):
                                accap = accs[ti][:]
                                for j in range(C_sub):
                                    k = ti * C_sub + j
                                    nc.tensor.matmul(
                                        out=accap,
                                        lhsT=reg[:, k, :],
                                        rhs=st[:, :, k],
                                        start=(c == 0 and j == 0),
                                        stop=(c == NCH - 1 and j == C_sub - 1),
                                    )
                        else:
                            st = stp.tile([P, gc, P], f16, tag="st")
                            nc.vector.tensor_tensor(
                                out=st[:],
                                in0=iota_sb[:, :, 0].unsqueeze(1)
                                    .to_broadcast([P, gc, P]),
                                in1=rel_sb[:, c, t0 * C_sub:t0 * C_sub + gc]
                                    .unsqueeze(2).to_broadcast([P, gc, P]),
                                op=eq,
                            )
                            for ti in range(gt):
                                accap = accs[ti][:]
                                for j in range(C_sub):
                                    k = ti * C_sub + j
                                    nc.tensor.matmul(
                                        out=accap,
                                        lhsT=reg[:, k, :],
                                        rhs=st[:, k, :],
                                        start=(c == 0 and j == 0),
                                        stop=(c == NCH - 1 and j == C_sub - 1),
                                    )
                    for ti in range(gt):
                        t = t0 + ti
                        accap = accs[ti][:]
                        aggT = smallp.tile([P, P], f16, tag="agg")
                        nc.scalar.copy(aggT[:], accap)
                        # reuse the same PSUM bank for the output matmul
                        nc.tensor.matmul(out=accap, lhsT=w_sb[:], rhs=aggT[:],
                                         start=True, stop=False)
                        nc.tensor.matmul(out=accap, lhsT=b_sb[:1, :],
                                         rhs=deg_sb[:1, t * P:(t + 1) * P],
                                         start=False, stop=True)
                        osb = smallp.tile([P, P], f32, tag="osb")
                        nc.scalar.copy(osb[:], accap)
                        nc.sync.dma_start(
                            out_d.ap()[:, t * P:(t + 1) * P], osb[:])

            if repeat == 1:
                body()
            else:
                with tc.For_i(0, repeat, 1) as iv:
                    body(iv)

    nc.compile()
    return nc


def _run(cfg, x, row, col, W, b, repeat=1, core_ids=None):
    from concourse import bass_utils

    C_sub, in_maps, perm, invdeg = _host_prep(cfg, x, row, col, W, b)
    key = (cfg.NN, cfg.NCORES, C_sub, repeat)
    if key not in _BUILD_CACHE:
        _BUILD_CACHE[key] = _build(cfg, C_sub, repeat)
    nc = _BUILD_CACHE[key]
    if core_ids is None:
        core_ids = list(range(cfg.NCORES))
    res = bass_utils.run_bass_kernel_spmd(nc, in_maps, core_ids=core_ids)
    outs = []
    for c in range(len(core_ids)):
        outT = res.results[c]["outT"]
        dsl = slice(c * cfg.NDEST, (c + 1) * cfg.NDEST)
        outs.append(outT.T[perm[c]] * invdeg[dsl][:, None])
    return np.concatenate(outs, axis=0)


def kernel(x, row, col, W, b):
    return _run(CFG, x, row, col, W, b, repeat=1)


# revision 29
# speedup vs baseline: 1.8295x; 1.3188x over previous
# GNN mean-aggregation kernel for Trainium2 (8 NeuronCores, SPMD).
#
# Computes: out[i] = (1/deg_i) * sum_{(i,j) in E} (x[j] @ W + b)
# using the identity  out = inv_deg * ((A @ x) @ W + b (x) deg),
# so the dense linear layer runs on the 100k aggregated rows instead of
# per-edge features.
#
# Sharding: destination nodes (and their incoming edge rows -- `row` is
# sorted) are split contiguously across 8 cores; x and W are replicated,
# so no collectives are needed.
#
# v2 layout (all fp16 on the wide paths; PSUM accumulates fp32):
#   1. dma_gather (GPSIMD SWDGE) fetches x_fp16[col] rows (256B each) from
#      HBM, one call per (8-tile dest group, 25k-src chunk) to amortize the
#      ~1us fixed SWDGE cost.  int16 gather indices only span 32k rows, so
#      x is addressed in 4 chunks of 25k rows and edges are host-binned by
#      (dest-tile, chunk), padded to C_sub*128 per bin.
#   2. DVE builds one-hot segment matrices S[e, d, j] = (d == rel[e, j]) in
#      fp16 with the block index j packed LAST so the 16-bit 2x DVE mode
#      applies (one instruction per group-chunk).
#   3. PE accumulates AGG^T = sum_j M_j^T @ S_j in PSUM per 128-dest tile,
#      then OUT^T = W^T @ AGG^T + b (x) deg  (rank-1 bias matmul).
#   4. OUT^T tiles are DMA'd straight from PSUM to HBM in fp32.
# Host post-processing transposes, un-permutes, and scales by inv_deg.

import math

import numpy as np

P = 128
F = 128


class _Cfg:
    def __init__(self, n_nodes, n_cores, n_chunks, group_tiles=8):
        self.NN = n_nodes
        self.NCORES = n_cores
        self.NDEST = n_nodes // n_cores
        self.NT = math.ceil(self.NDEST / P)
        self.NCH = n_chunks
        self.CH = math.ceil(n_nodes / n_chunks)
        assert self.CH <= 32768
        # x rows are laid out in NCH chunks of CHX >= CH rows (tail rows
        # unused) so chunk node-capacity never binds during balancing.
        self.CHX = min(32000, self.CH + max(64, self.CH // 5))
        self.XROWS = n_chunks * self.CHX
        self.G = group_tiles


CFG = _Cfg(100000, 8, 4)

_BUILD_CACHE = {}


def _balance_chunks(cfg, core, col):
    """Permute source nodes so each (core, chunk) edge count is ~equal.

    Returns src_pos[j] = permuted position of source node j in the padded
    [NCH * CHX] x layout.  Chunk of an edge becomes src_pos[col] // CHX.
    """
    NN, NCORES, NCH, CH = cfg.NN, cfg.NCORES, cfg.NCH, cfg.CHX
    M = np.zeros((NN, NCORES), np.int32)
    np.add.at(M, (col, core), 1)
    order = np.argsort(-M.sum(axis=1), kind="stable")
    Mo = M[order].tolist()
    loads = [[0] * NCORES for _ in range(NCH)]
    counts = [0] * NCH
    choice = np.empty(NN, np.int64)
    # greedy squared-loss: assign each node (largest first) to the chunk
    # minimizing sum of squared per-core loads
    for i in range(NN):
        v = Mo[i]
        bestk, bestpen = -1, None
        for k in range(NCH):
            if counts[k] >= CH:
                continue
            lk = loads[k]
            pen = 0
            for j in range(NCORES):
                nl = lk[j] + v[j]
                pen += nl * nl
            if bestpen is None or pen < bestpen:
                bestk, bestpen = k, pen
        choice[i] = bestk
        lk = loads[bestk]
        for j in range(NCORES):
            lk[j] += v[j]
        counts[bestk] += 1
    # Move-refinement: flatten per-(chunk, core) cells toward the mean so
    # the downstream per-tile cap has slack on every core.
    loads = np.array(loads, np.int64)
    counts = np.array(counts, np.int64)
    node_of = choice[np.argsort(order)]  # chunk of original node id
    rng = np.random.default_rng(0)
    tgt = int(loads.sum(axis=0).max() // NCH + 110)
    members = [list(np.where(node_of == k)[0]) for k in range(NCH)]
    for _ in range(3000):
        k, j = np.unravel_index(np.argmax(loads), loads.shape)
        if loads[k, j] <= tgt:
            break
        k2 = int(np.argmin(loads[:, j] + np.where(counts < CH, 0, 1 << 40)))
        if k2 == k:
            break
        mem = np.array(members[k])
        cand = mem[rng.integers(0, len(mem), 512)]
        v = M[cand]
        # prefer nodes with many j-edges and few edges for already-hot cores
        gain = v[:, j].astype(np.int64) * 4 - v.sum(axis=1)
        best = cand[int(np.argmax(gain))]
        if M[best, j] == 0:
            continue
        loads[k] -= M[best]
        loads[k2] += M[best]
        counts[k] -= 1
        counts[k2] += 1
        members[k].remove(best)
        members[k2].append(best)
        node_of[best] = k2
    src_pos = np.empty(NN, np.int64)
    for k in range(NCH):
        nodes = np.where(node_of == k)[0]
        src_pos[nodes] = k * CH + np.arange(len(nodes))
    return src_pos


def _refine_tiles(cnt, tile_of, sums, counts, cap, max_iter=8000, seed=0):
    """Swap/move dests between tiles until every (tile, chunk) bin <= cap."""
    rng = np.random.default_rng(seed)
    NT, NCH = sums.shape
    PCAP = 128
    members = [list(np.where(tile_of == t)[0]) for t in range(NT)]
    stall = 0
    best = None
    best_tot = 1 << 30
    for it in range(max_iter):
        over = np.maximum(sums - cap, 0)
        tot = int(over.sum())
        if tot == 0:
            return True
        if tot < best_tot:
            best_tot, stall = tot, 0
            best = (tile_of.copy(), sums.copy(), counts.copy(),
                    [m[:] for m in members])
        else:
            stall += 1
        if stall > 300:
            tile_of[:], sums[:], counts[:] = best[0], best[1], best[2]
            members = [m[:] for m in best[3]]
            hot = np.argsort(-sums.max(axis=1))[:10]
            for _ in range(4):
                t_h = int(rng.choice(hot))
                t_r = int(rng.integers(0, NT))
                if t_r == t_h or not members[t_h] or not members[t_r]:
                    continue
                dd = int(rng.choice(members[t_h]))
                d2 = int(rng.choice(members[t_r]))
                v, w = cnt[dd], cnt[d2]
                sums[t_h] += w - v
                sums[t_r] += v - w
                members[t_h].remove(dd)
                members[t_h].append(d2)
                members[t_r].remove(d2)
                members[t_r].append(dd)
                tile_of[dd] = t_r
                tile_of[d2] = t_h
            stall = 0
            continue
        t, cstar = np.unravel_index(np.argmax(sums), sums.shape)
        mem = np.array(members[t])
        wide = tot <= 32
        dcands = mem if wide else mem[np.argsort(-cnt[mem, cstar])[:10]]
        V = cnt[dcands]
        tcands = (np.arange(NT) if wide
                  else np.argsort(sums[:, cstar])[:16])
        tcands = tcands[tcands != t]
        pen_t = over[t].sum()
        bestmv = None
        for t2 in tcands:
            mem2 = np.array(members[t2])
            pen_old = pen_t + over[t2].sum()
            if counts[t2] < PCAP:
                ns_t = sums[t][None, :] - V
                ns_t2 = sums[t2][None, :] + V
                pen_new = (np.maximum(ns_t - cap, 0).sum(axis=1)
                           + np.maximum(ns_t2 - cap, 0).sum(axis=1))
                gain = pen_old - pen_new
                gi = int(np.argmax(gain))
                if gain[gi] > 0 and (bestmv is None or gain[gi] > bestmv[0]):
                    bestmv = (gain[gi], dcands[gi], None, t2)
            d2c = mem2[np.argsort(cnt[mem2, cstar])[:10]]
            Wm = cnt[d2c]
            ns_t = sums[t][None, None, :] - V[:, None, :] + Wm[None, :, :]
            ns_t2 = sums[t2][None, None, :] + V[:, None, :] - Wm[None, :, :]
            pen_new = (np.maximum(ns_t - cap, 0).sum(axis=2)
                       + np.maximum(ns_t2 - cap, 0).sum(axis=2))
            gain = pen_old - pen_new
            gi = np.unravel_index(np.argmax(gain), gain.shape)
            if gain[gi] > 0 and (bestmv is None or gain[gi] > bestmv[0]):
                bestmv = (gain[gi], dcands[gi[0]], d2c[gi[1]], t2)
        if bestmv is None:
            stall += 30
            continue
        _, dd, d2, t2 = bestmv
        if d2 is None:
            sums[t] -= cnt[dd]
            sums[t2] += cnt[dd]
            counts[t] -= 1
            counts[t2] += 1
            members[t].remove(dd)
            members[t2].append(dd)
            tile_of[dd] = t2
        else:
            v, w = cnt[dd], cnt[d2]
            sums[t] += w - v
            sums[t2] += v - w
            members[t].remove(dd)
            members[t].append(d2)
            members[t2].remove(d2)
            members[t2].append(dd)
            tile_of[dd] = t2
            tile_of[d2] = t
    if best is not None:
        tile_of[:], sums[:], counts[:] = best[0], best[1], best[2]
    return False


def _balance_cores(cfg, deg):
    """Assign dests to cores so per-core edge totals are ~equal.

    Returns dest_core[d] and dest_ids[c] (sorted dest ids of core c).
    """
    NN, NCORES, NDEST = cfg.NN, cfg.NCORES, cfg.NDEST
    order = np.argsort(-deg, kind="stable")
    degs = deg[order].tolist()
    loads = [0.0] * NCORES
    counts = [0] * NCORES
    dest_core = np.empty(NN, np.int64)
    for i in range(NN):
        bestk, bestv = -1, None
        for k in range(NCORES):
            if counts[k] >= NDEST:
                continue
            if bestv is None or loads[k] < bestv:
                bestk, bestv = k, loads[k]
        dest_core[order[i]] = bestk
        loads[bestk] += degs[i]
        counts[bestk] += 1
    dest_ids = [np.where(dest_core == c)[0] for c in range(NCORES)]
    return dest_core, dest_ids


def _host_prep(cfg, x, row, col, W, b):
    NN, NCORES, NDEST, NT, NCH, CH = (
        cfg.NN, cfg.NCORES, cfg.NDEST, cfg.NT, cfg.NCH, cfg.CH)
    NE = row.shape[0]
    row = np.asarray(row).astype(np.int64)
    col = np.asarray(col).astype(np.int64)
    x = np.ascontiguousarray(np.asarray(x, dtype=np.float32))
    W = np.ascontiguousarray(np.asarray(W, dtype=np.float32))
    b = np.asarray(b, dtype=np.float32)

    deg = np.bincount(row, minlength=NN).astype(np.float32)
    invdeg = np.where(deg > 0, 1.0 / np.maximum(deg, 1.0), 0.0).astype(np.float32)

    # Dest->core assignment balancing per-core edge totals; r_in_core is
    # the dest's rank within its core's sorted id list.
    dest_core, dest_ids = _balance_cores(cfg, deg)
    dest_rank = np.empty(NN, np.int64)
    for c in range(NCORES):
        dest_rank[dest_ids[c]] = np.arange(len(dest_ids[c]))
    core = dest_core[row]
    r_in_core = dest_rank[row]

    # Source permutation: balance per-(core, chunk) edge totals so the
    # per-tile bin cap of 8*128 is feasible on every core.
    src_pos = _balance_chunks(cfg, core, col)
    x_perm = np.zeros((cfg.XROWS, x.shape[1]), x.dtype)
    x_perm[src_pos] = x
    pcol = src_pos[col]
    chunk = pcol // cfg.CHX
    idx16 = (pcol % cfg.CHX).astype(np.int16)

    # Dest->tile assignment: greedy, then local-search refinement toward
    # per-(tile, chunk) bins <= 8*128 (falls back to whatever max it hits).
    perm = np.zeros((NCORES, NDEST), np.int64)
    cap = 8 * P
    ok_all = True
    for c in range(NCORES):
        sel = core == c
        cnt = np.zeros((NDEST, NCH), np.int32)
        np.add.at(cnt, (r_in_core[sel], chunk[sel]), 1)
        order_d = np.argsort(-cnt.max(axis=1), kind="stable")
        sums = np.zeros((NT, NCH), np.int32)
        counts = np.zeros(NT, np.int32)
        tile_of = np.empty(NDEST, np.int64)
        big = np.int32(1 << 30)
        for d in order_d:
            newmax = np.maximum(sums, cnt[d]).max(axis=1)
            t = int(np.argmin(np.where(counts < P, newmax, big)))
            tile_of[d] = t
            counts[t] += 1
            sums[t] += cnt[d]
        if sums.max() > cap:
            ok_all &= _refine_tiles(cnt, tile_of, sums, counts, cap, seed=c)
        # slot positions within each tile
        pos = np.empty(NDEST, np.int64)
        fill = np.zeros(NT, np.int64)
        for d in range(NDEST):
            t = tile_of[d]
            pos[d] = t * P + fill[t]
            fill[t] += 1
        perm[c] = pos
    tilei = perm[core, r_in_core] // P
    rel = (perm[core, r_in_core] % P).astype(np.float32)
    return _host_prep_finish(
        cfg, x_perm, W, b, deg, invdeg, core, chunk, idx16, tilei, rel, perm,
        dest_ids)


def _host_prep_finish(cfg, x, W, b, deg, invdeg, core, chunk, idx16,
                      tilei, rel, perm, dest_ids):
    NN, NCORES, NDEST, NT, NCH, CH, G = (
        cfg.NN, cfg.NCORES, cfg.NDEST, cfg.NT, cfg.NCH, cfg.CH, cfg.G)
    NE = core.shape[0]
    bin_key = (core * NT + tilei) * NCH + chunk
    nbins = NCORES * NT * NCH
    counts = np.bincount(bin_key, minlength=nbins)
    C_sub = max(1, int(math.ceil(counts.max() / P)))
    SLOT = C_sub * P

    # Within each bin, order gathers by ascending source index so each
    # SWDGE call walks HBM mostly monotonically (row-buffer friendly).
    order = np.lexsort((idx16, bin_key))
    sk = bin_key[order]
    starts = np.concatenate([[0], np.cumsum(counts)[:-1]])
    rank = np.arange(NE, dtype=np.int64) - starts[sk]
    pos = sk * SLOT + rank

    TOT = nbins * SLOT
    idx_pad = np.zeros(TOT, np.int16)
    rel_pad = np.full(TOT, -1.0, np.float32)
    idx_pad[pos] = idx16[order]
    rel_pad[pos] = rel[order]
    # Pad slots repeat the bin's last real index (same-address reads are
    # nearly free); empty bins keep index 0.
    slot_grid = idx_pad.reshape(nbins, SLOT)
    fill = np.zeros(nbins, np.int16)
    np.maximum.at(fill, sk, idx16[order])
    pad_mask = rel_pad.reshape(nbins, SLOT) < 0
    slot_grid[pad_mask] = np.broadcast_to(fill[:, None], (nbins, SLOT))[pad_mask]
    # [core, tile, chunk, C_sub, P] (slot s = (j, p) = (s // P, s % P))
    idx_pad = idx_pad.reshape(NCORES, NT, NCH, SLOT)
    rel_pad = rel_pad.reshape(NCORES, NT, NCH, C_sub, P)

    groups = [(t0, min(t0 + G, NT)) for t0 in range(0, NT, G)]

    x16 = x.astype(np.float16)
    W16 = W.astype(np.float16)
    b16 = b.astype(np.float16)[None, :]
    iota3 = np.tile(
        np.arange(P, dtype=np.float16)[None, :, None], (P, 1, G * C_sub)
    ).reshape(P, P * G * C_sub)

    in_maps = []
    for c in range(NCORES):
        # gather-call index stream: one call per (group, chunk), wrapped in
        # 16 partitions and replicated x8 down the partition dim.
        wrapped_parts = []
        for (t0, t1) in groups:
            for ch in range(NCH):
                seq = idx_pad[c, t0:t1, ch].reshape(-1)
                wrapped_parts.append(np.tile(seq.reshape(-1, 16).T, (8, 1)))
        idx_t = np.concatenate(wrapped_parts, axis=1)

        # rel2[p, ch, t*C_sub + j] = rel_pad[c, t, ch, j, p]
        rel2 = np.ascontiguousarray(
            rel_pad[c].transpose(3, 1, 0, 2).reshape(P, NCH, NT * C_sub)
        ).astype(np.float16).reshape(P, NCH * NT * C_sub)

        dgc = np.zeros(NT * P, np.float32)
        dgc[perm[c]] = deg[dest_ids[c]]

        in_maps.append({
            "x": x16,
            "idxs": np.ascontiguousarray(idx_t),
            "rel": rel2,
            "degr": dgc.astype(np.float16)[None, :],
            "w": W16,
            "brow": b16,
            "iota3": iota3,
        })
    return C_sub, in_maps, perm, invdeg, dest_ids


def _build(cfg, C_sub, repeat, onehot="edj", call="wrap",
           parts=("gather", "onehot", "mm", "tail"), gdt="f16"):
    import concourse.mybir as mybir
    import concourse.tile as tile
    from concourse import bacc

    f32 = mybir.dt.float32
    f16 = mybir.dt.float16
    i16 = mybir.dt.int16
    eq = mybir.AluOpType.is_equal
    gdtype = f32 if gdt == "f32" else f16

    NT, NCH, CH, G = cfg.NT, cfg.NCH, cfg.CH, cfg.G
    GC = G * C_sub
    IDXW = NT * NCH * C_sub * P // 16

    # sub = max indices per dma_gather call; the SWDGE descriptor ring
    # (dynamic_dma_scratch_size // 16 entries) must hold a full call.
    if call == "wrap":
        sub = 1024
    elif call == "big":
        sub = GC * P
    else:
        sub = int(call[1:])
    scratch = max(16384, 16 * sub)

    nc = bacc.Bacc("TRN2", debug=False, num_swdge_queues=4,
                   dynamic_dma_scratch_size=scratch)
    x_d = nc.dram_tensor("x", [cfg.XROWS, F], gdtype, kind="ExternalInput")
    idx_d = nc.dram_tensor("idxs", [P, IDXW], i16, kind="ExternalInput")
    rel_d = nc.dram_tensor("rel", [P, NCH * NT * C_sub], f16,
                           kind="ExternalInput")
    deg_d = nc.dram_tensor("degr", [1, NT * P], f16, kind="ExternalInput")
    w_d = nc.dram_tensor("w", [F, F], f16, kind="ExternalInput")
    b_d = nc.dram_tensor("brow", [1, F], f16, kind="ExternalInput")
    iota_d = nc.dram_tensor("iota3", [P, P * GC], f16, kind="ExternalInput")
    out_d = nc.dram_tensor("outT", [P, NT * P], f32, kind="ExternalOutput")

    groups = [(t0, min(t0 + G, NT)) for t0 in range(0, NT, G)]
    x_ap = x_d.ap()

    with tile.TileContext(nc) as tc:
        with (
            tc.tile_pool(name="const", bufs=1) as constp,
            tc.tile_pool(name="reg", bufs=3) as regionp,
            tc.tile_pool(name="st", bufs=2) as stp,
            tc.tile_pool(name="idx", bufs=3) as idxp,
            tc.tile_pool(name="small", bufs=4) as smallp,
            tc.tile_pool(name="acc", bufs=8, space="PSUM") as accp,
        ):
            w_sb = constp.tile([F, F], f16)
            nc.sync.dma_start(w_sb[:], w_d.ap())
            b_sb = constp.tile([1, F], f16)
            nc.sync.dma_start(b_sb[:], b_d.ap())
            iota_sb = constp.tile([P, P, GC], f16)
            nc.sync.dma_start(iota_sb[:], iota_d.ap())
            rel_sb = constp.tile([P, NCH, NT * C_sub], f16)
            nc.sync.dma_start(rel_sb[:], rel_d.ap())
            deg_sb = constp.tile([1, NT * P], f16)
            nc.sync.dma_start(deg_sb[:], deg_d.ap())

            def body(_iv=None):
                idx_off = 0
                qn = 0
                for (t0, t1) in groups:
                    gt = t1 - t0
                    gc = gt * C_sub
                    accs = [
                        accp.tile([P, P], f32, tag="acc", name=f"acc{t0}_{k}")
                        for k in range(gt)
                    ]
                    for c in range(NCH):
                        L = gc * P
                        reg = regionp.tile([P, gc, P], gdtype, tag="reg")
                        idxt = idxp.tile([P, gc * 8], i16, tag="idx")
                        nc.sync.dma_start(
                            idxt[:], idx_d.ap()[:, idx_off:idx_off + gc * 8])
                        idx_off += gc * 8
                        in_ap = x_ap[c * cfg.CHX:(c + 1) * cfg.CHX, :]
                        kstep = sub // P
                        for k0 in range(0, gc, kstep) if "gather" in parts else []:
                            kc = min(kstep, gc - k0)
                            Ls = kc * P
                            nc.gpsimd.dma_gather(
                                out_ap=reg[:, k0:k0 + kc, :],
                                in_ap=in_ap,
                                idxs_ap=idxt[:, k0 * 8:k0 * 8 + kc * 8],
                                num_idxs=Ls,
                                num_idxs_reg=Ls,
                                elem_size=F,
                                queue_num=qn % 4,
                            )
                            qn += 1
                        if "onehot" not in parts and "mm" not in parts:
                            continue
                        if onehot == "edj":
                            st = stp.tile([P, P, gc], f16, tag="st")
                            nc.vector.tensor_tensor(
                                out=st[:],
                                in0=iota_sb[:, :, :gc],
                                in1=rel_sb[:, c, t0 * C_sub:t0 * C_sub + gc]
                                    .unsqueeze(1).to_broadcast([P, P, gc]),
                                op=eq,
                            )
                            for ti in range(gt) if "mm" in parts else []:
                                accap = accs[ti][:]
                                for j in range(C_sub):
                                    k = ti * C_sub + j
                                    nc.tensor.matmul(
                                        out=accap,
                                        lhsT=reg[:, k, :],
                                        rhs=st[:, :, k],
                                        start=(c == 0 and j == 0),
                                        stop=(c == NCH - 1 and j == C_sub - 1),
                                    )
                        else:
                            st = stp.tile([P, gc, P], f16, tag="st")
                            nc.vector.tensor_tensor(
                                out=st[:],
                                in0=iota_sb[:, :, 0].unsqueeze(1)
                                    .to_broadcast([P, gc, P]),
                                in1=rel_sb[:, c, t0 * C_sub:t0 * C_sub + gc]
                                    .unsqueeze(2).to_broadcast([P, gc, P]),
                                op=eq,
                            )
                            for ti in range(gt) if "mm" in parts else []:
                                accap = accs[ti][:]
                                for j in range(C_sub):
                                    k = ti * C_sub + j
                                    nc.tensor.matmul(
                                        out=accap,
                                        lhsT=reg[:, k, :],
                                        rhs=st[:, k, :],
                                        start=(c == 0 and j == 0),
                                        stop=(c == NCH - 1 and j == C_sub - 1),
                                    )
                    for ti in range(gt) if "tail" in parts and "mm" in parts else []:
                        t = t0 + ti
                        accap = accs[ti][:]
                        aggT = smallp.tile([P, P], f16, tag="agg")
                        nc.scalar.copy(aggT[:], accap)
                        # reuse the same PSUM bank for the output matmul
                        nc.tensor.matmul(out=accap, lhsT=w_sb[:], rhs=aggT[:],
                                         start=True, stop=False)
                        nc.tensor.matmul(out=accap, lhsT=b_sb[:1, :],
                                         rhs=deg_sb[:1, t * P:(t + 1) * P],
                                         start=False, stop=True)
                        osb = smallp.tile([P, P], f32, tag="osb")
                        nc.scalar.copy(osb[:], accap)
                        nc.sync.dma_start(
                            out_d.ap()[:, t * P:(t + 1) * P], osb[:])

            if repeat == 1:
                body()
            else:
                with tc.For_i(0, repeat, 1) as iv:
                    body(iv)

    nc.compile()
    return nc


def _run(cfg, x, row, col, W, b, repeat=1, core_ids=None):
    from concourse import bass_utils

    C_sub, in_maps, perm, invdeg, dest_ids = _host_prep(cfg, x, row, col, W, b)
    key = (cfg.NN, cfg.NCORES, C_sub, repeat)
    if key not in _BUILD_CACHE:
        _BUILD_CACHE[key] = _build(cfg, C_sub, repeat)
    nc = _BUILD_CACHE[key]
    if core_ids is None:
        core_ids = list(range(cfg.NCORES))
    res = bass_utils.run_bass_kernel_spmd(nc, in_maps, core_ids=core_ids)
    out = np.empty((cfg.NN, F), np.float32)
    for c in range(len(core_ids)):
        outT = res.results[c]["outT"]
        ids = dest_ids[c]
        out[ids] = outT.T[perm[c]] * invdeg[ids][:, None]
    return out


def kernel(x, row, col, W, b):
    return _run(CFG, x, row, col, W, b, repeat=1)
